# revision 7
# baseline (speedup 1.0000x reference)
"""AttGRU Trainium2 kernel: attention scores + softmax + GRU-style scan.

Sharding: data-parallel over batch (512 -> 8 cores x 64).
Layout inside each core: "transposed" [H(partitions), B(free)] so the
recurrence h' needs no per-step transposes:
  - state hT [101, 64] in SBUF (row 100 == 1.0 to fold biases),
  - per-step matmuls: lhsT = augmented weight [101, 100], rhs = hT.
  - context-side matmuls (Wr@c, W@c) are batched per 8-step chunk into
    PSUM banks; the recurrent Ur@h accumulates into the same bank slice.
  - gate g_s is broadcast across partitions with a K=1 ones-matmul.
"""

import functools
import os

import numpy as np

B, S, H = 512, 512, 100
NCORES = 8
BL = B // NCORES  # 64 batch per core
HA = H + 1  # augmented hidden (ones row)
CH = 8  # scan steps per chunk
NCHUNK = S // CH
P1CH = 128  # pass-1 (attention) s-chunk


def _build_nc():
    import concourse.bacc as bacc
    import concourse.tile as tile
    from concourse import mybir

    fp32 = mybir.dt.float32
    AF = mybir.ActivationFunctionType

    nc = bacc.Bacc(
        "TRN2", target_bir_lowering=False, debug=False, num_devices=NCORES
    )

    # ---------------- DRAM I/O ----------------
    ctx_d = nc.dram_tensor("ctx_T", (S, HA, BL), fp32, kind="ExternalInput").ap()
    qT_d = nc.dram_tensor("qT", (H, BL), fp32, kind="ExternalInput").ap()
    h0_d = nc.dram_tensor("h0T", (HA, BL), fp32, kind="ExternalInput").ap()
    WrT_d = nc.dram_tensor("WrT_a", (HA, H), fp32, kind="ExternalInput").ap()
    WT_d = nc.dram_tensor("WT_a", (HA, H), fp32, kind="ExternalInput").ap()
    UrT_d = nc.dram_tensor("UrT_a", (HA, H), fp32, kind="ExternalInput").ap()
    UT_d = nc.dram_tensor("UT_a", (HA, H), fp32, kind="ExternalInput").ap()
    sel_d = nc.dram_tensor("sel_w", (128, 128 * H), fp32, kind="ExternalInput").ap()
    id_d = nc.dram_tensor("ident", (128, 128), fp32, kind="ExternalInput").ap()

    outh_d = nc.dram_tensor("out_hT", (H, BL), fp32, kind="ExternalOutput").ap()
    scr_d = nc.dram_tensor("scores_o", (BL, S), fp32, kind="ExternalOutput").ap()

    with tile.TileContext(nc) as tc:
        with tc.tile_pool(name="const", bufs=1) as cpool:
            WrT = cpool.tile([HA, H], fp32, name="WrT")
            WT = cpool.tile([HA, H], fp32, name="WT")
            UrT = cpool.tile([HA, H], fp32, name="UrT")
            UT = cpool.tile([HA, H], fp32, name="UT")
            selw = cpool.tile([128, 128 * H], fp32, name="selw")
            ident = cpool.tile([128, 128], fp32, name="ident")
            qTs = cpool.tile([H, BL], fp32, name="qTs")
            nc.sync.dma_start(WrT[:], WrT_d[:])
            nc.sync.dma_start(WT[:], WT_d[:])
            nc.sync.dma_start(UrT[:], UrT_d[:])
            nc.sync.dma_start(UT[:], UT_d[:])
            nc.sync.dma_start(selw[:], sel_d[:])
            nc.sync.dma_start(ident[:], id_d[:])
            nc.sync.dma_start(qTs[:], qT_d[:])

            # persistent softmax / scores tiles
            attn = cpool.tile([BL, S], fp32, name="attn")
            scores = cpool.tile([BL, S], fp32, name="scores")
            scT = cpool.tile([128, 4, BL], fp32, name="scT")  # [s%128, s//128, b]

            # ---------------- pass 1: attention + softmax ----------------
            with (
                tc.tile_pool(name="p1ctx", bufs=2) as p1pool,
                tc.tile_pool(name="p1ps", bufs=2, space="PSUM") as p1ps,
                tc.tile_pool(name="p1sb", bufs=2) as p1sb,
            ):
                for st in range(S // P1CH):
                    cchunk = p1pool.tile([HA, P1CH, BL], fp32, name="cchunk")
                    nc.sync.dma_start(
                        cchunk[:],
                        ctx_d[st * P1CH : (st + 1) * P1CH].rearrange(
                            "s d b -> d s b"
                        ),
                    )
                    aps = p1ps.tile([P1CH, BL], fp32, name="aps")
                    for b in range(BL):
                        # lhsT: [d(100part), s(128)] strided view at batch b
                        nc.tensor.matmul(
                            aps[:, b : b + 1],
                            cchunk[0:H, :, b],
                            qTs[:, b : b + 1],
                            start=True,
                            stop=True,
                        )
                    asb = p1sb.tile([P1CH, BL], fp32, name="asb")
                    nc.vector.tensor_copy(asb[:], aps[:])
                    atp = p1ps.tile([BL, P1CH], fp32, name="atp")
                    nc.tensor.transpose(atp[:], asb[:], ident[0:P1CH, 0:P1CH])
                    nc.vector.tensor_copy(
                        attn[:, st * P1CH : (st + 1) * P1CH], atp[:]
                    )

                # softmax over s (free dim) in [b, s] layout
                mx = p1sb.tile([BL, 1], fp32, name="mx")
                nc.vector.tensor_reduce(
                    mx[:], attn[:], mybir.AxisListType.X, mybir.AluOpType.max
                )
                nmx = p1sb.tile([BL, 1], fp32, name="nmx")
                nc.vector.tensor_scalar_mul(nmx[:], mx[:], -1.0)
                ssum = p1sb.tile([BL, 1], fp32, name="ssum")
                nc.scalar.activation(
                    scores[:], attn[:], AF.Exp, bias=nmx[:], accum_out=ssum[:]
                )
                rinv = p1sb.tile([BL, 1], fp32, name="rinv")
                nc.vector.reciprocal(rinv[:], ssum[:])
                nc.vector.tensor_scalar_mul(scores[:], scores[:], rinv[:])
                nc.sync.dma_start(scr_d[:], scores[:])

                # scoresT tiles for the gate-broadcast matmuls
                for st in range(S // 128):
                    stp = p1ps.tile([128, BL], fp32, name="stp")
                    nc.tensor.transpose(
                        stp[:], scores[:, st * 128 : (st + 1) * 128],
                        ident[0:BL, 0:BL],
                    )
                    nc.vector.tensor_copy(scT[:, st, :], stp[:])

            # ---------------- pass 2: the scan ----------------
            hA = nc.alloc_sbuf_tensor("hA", [HA, BL], fp32).ap()
            hB = nc.alloc_sbuf_tensor("hB", [HA, BL], fp32).ap()
            nc.sync.dma_start(hA[:], h0_d[:])
            nc.sync.dma_start(hB[:], h0_d[:])

            with (
                tc.tile_pool(name="ctxp", bufs=3) as ctxp,
                tc.tile_pool(name="qrp", bufs=2, space="PSUM") as qrp,
                tc.tile_pool(name="qxp", bufs=2, space="PSUM") as qxp,
                tc.tile_pool(name="gp", bufs=2, space="PSUM") as gp,
                tc.tile_pool(name="qup", bufs=2, space="PSUM") as qup,
                tc.tile_pool(name="ewp", bufs=3) as ewp,
            ):
                for kc in range(NCHUNK):
                    cch = ctxp.tile([HA, CH, BL], fp32, name="cch")
                    nc.sync.dma_start(
                        cch[:],
                        ctx_d[kc * CH : (kc + 1) * CH].rearrange("s d b -> d s b"),
                    )
                    qr = qrp.tile([H, CH, BL], fp32, name="qr")
                    qx = qxp.tile([H, CH, BL], fp32, name="qx")
                    gt = gp.tile([H, CH, BL], fp32, name="gt")
                    nc.tensor.matmul(
                        qr[:], WrT[:], cch[:], start=True, stop=False,
                        skip_group_check=True,
                    )
                    nc.tensor.matmul(qx[:], WT[:], cch[:], start=True, stop=True)
                    for i in range(CH):
                        s = kc * CH + i
                        r = s % 128
                        nc.tensor.matmul(
                            gt[:, i, :],
                            selw[:, r * H : (r + 1) * H],
                            scT[:, s // 128, :],
                            start=True,
                            stop=True,
                        )
                    for i in range(CH):
                        s = kc * CH + i
                        h_cur = hA if s % 2 == 0 else hB
                        h_nxt = hB if s % 2 == 0 else hA
                        qu = qup.tile([H, BL], fp32, name="qu")
                        nc.tensor.matmul(
                            qu[:], UT[:], h_cur[:], start=True, stop=True
                        )
                        nc.tensor.matmul(
                            qr[:, i, :], UrT[:], h_cur[:], start=False, stop=True,
                            skip_group_check=True,
                        )
                        r = ewp.tile([H, BL], fp32, name="r")
                        nc.scalar.activation(r[:], qr[:, i, :], AF.Sigmoid)
                        t1 = ewp.tile([H, BL], fp32, name="t1")
                        nc.vector.tensor_mul(t1[:], r[:], qu[:])
                        c2 = ewp.tile([H, BL], fp32, name="c2")
                        nc.vector.tensor_add(c2[:], t1[:], qx[:, i, :])
                        htl = ewp.tile([H, BL], fp32, name="htl")
                        nc.scalar.activation(htl[:], c2[:], AF.Tanh)
                        dd = ewp.tile([H, BL], fp32, name="dd")
                        nc.vector.tensor_sub(dd[:], htl[:], h_cur[0:H, :])
                        ee = ewp.tile([H, BL], fp32, name="ee")
                        nc.vector.tensor_mul(ee[:], dd[:], gt[:, i, :])
                        nc.vector.tensor_add(h_nxt[0:H, :], h_cur[0:H, :], ee[:])

            nc.sync.dma_start(outh_d[:], hA[0:H, :])

    nc.compile()
    return nc


@functools.lru_cache(maxsize=1)
def _get_nc():
    return _build_nc()


def _prep_inputs(query, context, init_hidden, Wr, bWr, Ur, bUr, W, bW, U, bU):
    f = np.float32
    query = np.asarray(query, f)
    context = np.asarray(context, f)
    init_hidden = np.asarray(init_hidden, f)
    WrT_a = np.concatenate([np.asarray(Wr, f).T, (np.asarray(bWr, f) + np.asarray(bUr, f))[None, :]], 0)
    WT_a = np.concatenate([np.asarray(W, f).T, np.asarray(bW, f)[None, :]], 0)
    UrT_a = np.concatenate([np.asarray(Ur, f).T, np.zeros((1, H), f)], 0)
    UT_a = np.concatenate([np.asarray(U, f).T, np.asarray(bU, f)[None, :]], 0)
    sel_w = np.repeat(np.eye(128, dtype=f), H, axis=1)
    ident = np.eye(128, dtype=f)

    in_maps = []
    for k in range(NCORES):
        sl = slice(k * BL, (k + 1) * BL)
        ctx_k = context[sl]  # (64, 512, 100)
        ctx_T = np.empty((S, HA, BL), f)
        ctx_T[:, :H, :] = ctx_k.transpose(1, 2, 0)
        ctx_T[:, H, :] = 1.0
        h0T = np.empty((HA, BL), f)
        h0T[:H] = init_hidden[0, sl].T
        h0T[H] = 1.0
        in_maps.append(
            {
                "ctx_T": np.ascontiguousarray(ctx_T),
                "qT": np.ascontiguousarray(query[sl, 0, :].T),
                "h0T": h0T,
                "WrT_a": WrT_a,
                "WT_a": WT_a,
                "UrT_a": UrT_a,
                "UT_a": UT_a,
                "sel_w": sel_w,
                "ident": ident,
            }
        )
    return in_maps


def kernel(query, context, init_hidden, Wr, bWr, Ur, bUr, W, bW, U, bU, _trace=False):
    from concourse import bass_utils

    nc = _get_nc()
    in_maps = _prep_inputs(
        query, context, init_hidden, Wr, bWr, Ur, bUr, W, bW, U, bU
    )
    res = bass_utils.run_bass_kernel_spmd(
        nc, in_maps, list(range(NCORES)), trace=_trace
    )
    out = np.empty((B, 1, H), np.float32)
    scores = np.empty((B, 1, S), np.float32)
    for k in range(NCORES):
        sl = slice(k * BL, (k + 1) * BL)
        out[sl, 0, :] = np.asarray(res.results[k]["out_hT"]).T
        scores[sl, 0, :] = np.asarray(res.results[k]["scores_o"])
    kernel._last_res = res
    return out, scores
